# revision 7
# baseline (speedup 1.0000x reference)
"""TRN2 Bass kernel for nn_Attention_87308095193383 — v5.

Sharding: 8 cores = (batch b in 0..3) x (query-half h in 0..1), SPMD with
host-permuted columns so "my queries" are always columns 0:1024.

Key structure (single core):
  A: conv1/conv2 (pe streamed nch-major) + GroupNorm (bn_stats, group
     reduce via indicator matmuls, affine as DVE 2x tensor_scalar)
  C: z = p2^T p1 -> sigmoid -> pa [keys, queries] bf16
  D: qkv, nch-major; x chunks + qkv_w sections prefetched through the
     shared rotating "stream" pool (slot-release sems give free prefetch)
  E: per (nqb, head): scores (f32r, K=64) -> mul by pa IN-PLACE in PSUM
     (DVE; some groups via ACT-copy + GpSimd mul) -> exp from PSUM (ACT
     -> bf16 e2, tiles also in the stream pool) -> attn@v accumulated in
     [q-partitions, 65-free] bf16; reciprocal([128,1]) + tensor_scalar
     normalize; depth-2 software pipeline so PE never waits on exp.
     o -> DRAM -> XBAR transpose back as oT [c, q]; proj for block 0
     interleaved into E of block 1.
  F: proj (bf16) + bias, streaming per-(ot,nqb) output DMA.
"""
import numpy as np
import ml_dtypes

import concourse.bass as bass
import concourse.mybir as mybir
import concourse.tile as tile
from concourse import bacc
from concourse.bass_utils import run_bass_kernel_spmd

F32R = mybir.dt.float32r
F32 = mybir.dt.float32
BF16 = mybir.dt.bfloat16
AF = mybir.ActivationFunctionType
ALU = mybir.AluOpType

N_CORES = 8
C = 512          # channels
CT = C // 128    # 4 c-tiles
N = 2048         # sequence length
NT = N // 128    # 16 m-tiles
NQ = 1024        # queries per core
H = 8            # heads
D = 64           # head dim
SCALE = D ** -0.5
EPS = 1e-5

# every k-th score group takes the ACT-copy + GpSimd-mul route
C_EVERY = 5


def build():
    nc = bacc.Bacc("TRN2", target_bir_lowering=False, debug=False,
                   num_devices=N_CORES)

    def din(name, shape, dt=F32R):
        return nc.dram_tensor(name, shape, dt, kind="ExternalInput").ap()

    peT = din("peT", [C, N])
    xT = din("xT", [C, N])
    cw1 = din("cw1", [C, C])        # conv1_w.T  [c_in, o]
    cw2 = din("cw2", [C, C])
    qw = din("qw", [C, 3 * C])      # qkv_w.T    [c_in, o]
    pw = din("pw", [C, C], BF16)    # proj_w.T
    cb1 = din("cb1", [C], F32)
    cb2 = din("cb2", [C], F32)
    gn1g = din("gn1g", [C], F32)
    gn1b = din("gn1b", [C], F32)
    gn2g = din("gn2g", [C], F32)
    gn2b = din("gn2b", [C], F32)
    pb = din("pb", [C], F32)
    gmask_in = din("gmask", [128, 2], F32)
    gmaskT_in = din("gmaskT", [2, 128], F32)
    vones_in = din("vones", [128, NT * H], BF16)
    outT = nc.dram_tensor("outT", [C, NQ], F32, kind="ExternalOutput").ap()

    with tile.TileContext(nc) as tc:
        _build_body(nc, tc, peT, xT, cw1, cw2, qw, pw, cb1, cb2,
                    gn1g, gn1b, gn2g, gn2b, pb, gmask_in, gmaskT_in,
                    vones_in, outT)
    nc.compile()
    return nc


def _build_body(nc, tc, peT, xT, cw1, cw2, qw, pw, cb1, cb2,
                gn1g, gn1b, gn2g, gn2b, pb, gmask_in, gmaskT_in,
                vones_in, outT):
    from contextlib import ExitStack
    ctx = ExitStack()
    with ctx:
        consts = ctx.enter_context(tc.tile_pool(name="consts", bufs=1))
        work = ctx.enter_context(tc.tile_pool(name="work", bufs=2))
        pa_pool = ctx.enter_context(tc.tile_pool(name="pa", bufs=1))
        pa = pa_pool.tile([128, NT, NQ], BF16)   # sigmoid(pe_attn) [k, q]
        pw_pool = ctx.enter_context(tc.tile_pool(name="pw_pool", bufs=1))
        pw_sb = pw_pool.tile([128, CT, C], BF16)
        nc.gpsimd.dma_start(pw_sb, pw.rearrange("(t p) o -> p t o", p=128))
        # shared rotating slots: pe chunks -> x chunks / qw sections -> e2
        stream = ctx.enter_context(tc.tile_pool(name="stream", bufs=6))
        dram_pool = ctx.enter_context(tc.tile_pool(name="dscr", bufs=1,
                                                   space="DRAM"))
        o_dram = dram_pool.tile([NQ, C], BF16)

        # ---- constants (scalar queue, needed from ~25us)
        gmask = consts.tile([128, 2], F32)     # group-membership mask
        nc.scalar.dma_start(gmask, gmask_in)
        gmaskT = consts.tile([2, 128], F32)
        nc.scalar.dma_start(gmaskT, gmaskT_in)
        epst = consts.tile([128, 1], F32)
        nc.vector.memset(epst, EPS)
        bias1 = consts.tile([128, CT], F32)
        nc.scalar.dma_start(bias1, cb1.rearrange("(t p) -> p t", p=128))
        bias2 = consts.tile([128, CT], F32)
        nc.scalar.dma_start(bias2, cb2.rearrange("(t p) -> p t", p=128))
        g1g = consts.tile([128, CT], F32)
        nc.scalar.dma_start(g1g, gn1g.rearrange("(t p) -> p t", p=128))
        g1b = consts.tile([128, CT], F32)
        nc.scalar.dma_start(g1b, gn1b.rearrange("(t p) -> p t", p=128))
        g2g = consts.tile([128, CT], F32)
        nc.scalar.dma_start(g2g, gn2g.rearrange("(t p) -> p t", p=128))
        g2b = consts.tile([128, CT], F32)
        nc.scalar.dma_start(g2b, gn2b.rearrange("(t p) -> p t", p=128))
        pbias = consts.tile([128, CT], F32)
        nc.scalar.dma_start(pbias, pb.rearrange("(t p) -> p t", p=128))

        # warm the Sqrt ACT-table set during the DMA wait (Copy is in
        # every set; Sigmoid/Exp sets get pre-triggered later)
        warmt = consts.tile([128, 2], F32)
        nc.vector.memset(warmt, 0.0)
        nc.scalar.activation(warmt[:, 0:1], warmt[:, 0:1], AF.Sqrt)

        # ================= stage A/B: conv + groupnorm =================
        ps_abcd = tc.tile_pool(name="ps_mm", bufs=6, space="PSUM")
        ps_mm = ps_abcd.__enter__()
        with tc.tile_pool(name="cw_pool", bufs=1) as cw_pool, \
             tc.tile_pool(name="p12", bufs=1) as p12_pool:
            cw1_sb = cw_pool.tile([128, CT, C], F32R)
            nc.gpsimd.dma_start(cw1_sb, cw1.rearrange("(t p) o -> p t o", p=128))
            cw2_sb = cw_pool.tile([128, CT, C], F32R)
            nc.gpsimd.dma_start(cw2_sb, cw2.rearrange("(t p) o -> p t o", p=128))
            pe_r = peT.rearrange("(t p) n -> p t n", p=128)

            # p1 only needs its first NQ columns kept; p2 needs all N.
            p1_sb = p12_pool.tile([128, CT, NQ], F32R)
            p2_sb = p12_pool.tile([128, CT, N], F32R)

            convs = [(cw1_sb, bias1, g1g, g1b, p1_sb, NQ),
                     (cw2_sb, bias2, g2g, g2b, p2_sb, N)]
            statss = []
            for conv_i in range(2):
                stats = work.tile([128, CT, N // 512, 6], F32,
                                  tag=f"gnstats{conv_i}")
                statss.append(stats)
            # conv-major: conv1's GN chain overlaps conv2's matmuls.
            pe_chs = []
            for nch in range(N // 512):
                pe_ch = stream.tile([128, CT, 512], F32R, tag="xch")
                nc.sync.dma_start(pe_ch, pe_r[:, :, nch * 512:(nch + 1) * 512])
                pe_chs.append(pe_ch)
            for conv_i, (cwsb, cbt, gg, gb, dst, keep) in enumerate(convs):
                for nch in range(N // 512):
                    for ot in range(CT):
                        ps = ps_mm.tile([128, 512], F32, tag="mm")
                        for ct in range(CT):
                            nc.tensor.matmul(
                                ps, cwsb[:, ct, ot * 128:(ot + 1) * 128],
                                pe_chs[nch][:, ct],
                                start=(ct == 0), stop=(ct == CT - 1))
                        nc.vector.bn_stats(statss[conv_i][:, ot, nch], ps)
                        if nch * 512 < keep:
                            # keep-copies on ACT (DVE busy with bn_stats)
                            nc.scalar.copy(dst[:, ot, nch * 512:(nch + 1) * 512], ps)

            # x chunks + qw sections claim slots as pe chunks retire; the
            # DMAs issue from sync/gpsimd SEQs within the first ~15us, so
            # everything stage D needs is resident before it starts.
            x_r = xT.rearrange("(t p) n -> p t n", p=128)
            qw_r = qw.rearrange("(t p) o -> p t o", p=128)
            x_chs = []
            qsecs = []
            preload = [("x", 0, nc.scalar), ("x", 1, nc.scalar),
                       ("q", 1, nc.gpsimd), ("q", 0, nc.scalar),
                       ("q", 2, nc.scalar), ("x", 2, nc.gpsimd),
                       ("x", 3, nc.gpsimd)]
            for kind, i, eng in preload:
                if kind == "x":
                    t_x = stream.tile([128, CT, 512], F32R, tag="xch")
                    eng.dma_start(t_x, x_r[:, :, i * 512:(i + 1) * 512])
                    x_chs.append((i, t_x))
                else:
                    t_q = stream.tile([128, CT, 512], F32R, tag="xch")
                    eng.dma_start(t_q, qw_r[:, :, i * 512:(i + 1) * 512])
                    qsecs.append((i, t_q))
            x_chs = [t for _, t in sorted(x_chs)]
            qsecs = [t for _, t in sorted(qsecs)]

            for conv_i, (cwsb, cbt, gg, gb, dst, keep) in enumerate(convs):
                stats = statss[conv_i]
                mv2 = work.tile([128, 2, CT], F32, tag="gnmv")
                stack3 = work.tile([128, 3, CT], F32, tag="gnstack")
                for ot in range(CT):
                    nc.vector.bn_aggr(mv2[:, :, ot], stats[:, ot])
                nc.vector.tensor_add(stack3[:, 0], mv2[:, 0], cbt)
                nc.vector.tensor_copy(stack3[:, 1], mv2[:, 1])
                nc.vector.tensor_mul(stack3[:, 2], stack3[:, 0], stack3[:, 0])
                # group sums over 64-partition halves (all ots at once)
                gs = ps_mm.tile([2, 3, CT], F32, tag="mm")
                nc.tensor.matmul(gs, gmask, stack3.rearrange("p a t -> p (a t)"),
                                 start=True, stop=True)
                gss = work.tile([2, 3, CT], F32, tag="gss")
                nc.scalar.copy(gss, gs)
                gstat = work.tile([2, 2, CT], F32, tag="gstat")  # [mean, rstd]
                nc.vector.tensor_scalar_mul(gstat[:, 0], gss[:, 0], 1.0 / 64.0)
                vt = work.tile([2, 2, CT], F32, tag="gvtmp")
                nc.vector.tensor_add(vt[:, 0], gss[:, 1], gss[:, 2])
                nc.vector.tensor_scalar_mul(vt[:, 0], vt[:, 0], 1.0 / 64.0)
                nc.vector.tensor_mul(vt[:, 1], gstat[:, 0], gstat[:, 0])
                nc.vector.tensor_sub(vt[:, 0], vt[:, 0], vt[:, 1])
                nc.scalar.activation(vt[:, 0], vt[:, 0], AF.Sqrt, bias=epst[0:2])
                nc.vector.reciprocal(gstat[:, 1], vt[:, 0])
                # broadcast group [mean, rstd] to partitions via indicator MM
                bc_ps = ps_mm.tile([128, 2, CT], F32, tag="mm")
                nc.tensor.matmul(bc_ps, gmaskT,
                                 gstat.rearrange("p a t -> p (a t)"),
                                 start=True, stop=True)
                bcst = work.tile([128, 2, CT], F32, tag="gbc")
                nc.scalar.copy(bcst, bc_ps)
                # per-channel affine: y = x*sc + sh
                sc = work.tile([128, 2, CT], F32, tag=f"gsc{conv_i}")
                nc.vector.tensor_mul(sc[:, 0], bcst[:, 1], gg)
                nc.vector.tensor_sub(sc[:, 1], cbt, bcst[:, 0])
                nc.vector.tensor_mul(sc[:, 1], sc[:, 1], sc[:, 0])
                nc.vector.tensor_add(sc[:, 1], sc[:, 1], gb)
                for nch in range(keep // 512):
                    for ot in range(CT):
                        # DVE all-SBUF tensor_scalar runs in 2x mode
                        nc.vector.tensor_scalar(
                            dst[:, ot, nch * 512:(nch + 1) * 512],
                            dst[:, ot, nch * 512:(nch + 1) * 512],
                            sc[:, 0, ot:ot + 1], sc[:, 1, ot:ot + 1],
                            op0=ALU.mult, op1=ALU.add)

            # pre-trigger the Sigmoid table-set load off the critical path
            nc.scalar.activation(warmt[:, 1:2], warmt[:, 1:2], AF.Sigmoid)

            # ================= stage C: pe_attn = sigmoid(p2^T p1) =====
            for mt in range(NT):
                for nq in range(NQ // 512):
                    zps = ps_mm.tile([128, 512], F32, tag="mm")
                    for ct in range(CT):
                        nc.tensor.matmul(
                            zps, p2_sb[:, ct, mt * 128:(mt + 1) * 128],
                            p1_sb[:, ct, nq * 512:(nq + 1) * 512],
                            start=(ct == 0), stop=(ct == CT - 1))
                    nc.scalar.activation(pa[:, mt, nq * 512:(nq + 1) * 512],
                                         zps, AF.Sigmoid)

        # ================= stage D: qkv =================
        kqv_pool = ctx.enter_context(tc.tile_pool(name="kqv", bufs=1))
        kT_sb = kqv_pool.tile([128, CT, N], F32R)
        qT_sb = kqv_pool.tile([128, CT, NQ], F32R)
        v_sb = kqv_pool.tile([128, NT, H, D + 1], BF16)
        nc.sync.dma_start(
            v_sb[:, :, :, D:D + 1].rearrange("p t o u -> p (t o u)"),
            vones_in)

        def emit_v(nch, pool, tag):
            xc = x_chs[nch]
            for ntl in range(4):
                nt = nch * 4 + ntl
                ps = pool.tile([128, 512], F32, tag=tag)
                for ct in range(CT):
                    nc.tensor.matmul(
                        ps, xc[:, ct, ntl * 128:(ntl + 1) * 128],
                        qsecs[2][:, ct],
                        start=(ct == 0), stop=(ct == CT - 1))
                nc.vector.tensor_copy(v_sb[:, nt, :, 0:D],
                                      ps.rearrange("p (h d) -> p h d", h=H))

        # nch-major kq; x chunks stay live for the v matmuls that are
        # interleaved into early stage E (v0 mid-D to unblock x3's slot)
        for nch in range(N // 512):
            xc = x_chs[nch]
            for ot in range(CT):
                ps = ps_mm.tile([128, 512], F32, tag="mm")
                for ct in range(CT):
                    nc.tensor.matmul(
                        ps, qsecs[1][:, ct, ot * 128:(ot + 1) * 128],
                        xc[:, ct],
                        start=(ct == 0), stop=(ct == CT - 1))
                # split kT copies between ACT and DVE
                if nch % 2 == 0:
                    nc.scalar.copy(kT_sb[:, ot, nch * 512:(nch + 1) * 512], ps)
                else:
                    nc.vector.tensor_copy(kT_sb[:, ot, nch * 512:(nch + 1) * 512], ps)
            if nch < NQ // 512:
                for ot in range(CT):
                    ps = ps_mm.tile([128, 512], F32, tag="mm")
                    for ct in range(CT):
                        nc.tensor.matmul(
                            ps, qsecs[0][:, ct, ot * 128:(ot + 1) * 128],
                            xc[:, ct],
                            start=(ct == 0), stop=(ct == CT - 1))
                    nc.vector.tensor_copy(qT_sb[:, ot, nch * 512:(nch + 1) * 512], ps)
            if nch == 1:
                emit_v(0, ps_mm, "mm")
        ps_abcd.__exit__(None, None, None)

        # ================= stage E: attention =================
        o_pool = ctx.enter_context(tc.tile_pool(name="opool", bufs=1))
        o_sb = o_pool.tile([128, NQ // 128, H, D], BF16)   # [q, qt, h, d]
        oT_sb = o_pool.tile([128, CT, NQ], BF16)           # [c, ct, q]
        fin_pool = ctx.enter_context(tc.tile_pool(name="finp", bufs=4))
        outT_r = outT.rearrange("(t p) n -> p t n", p=128)
        mulw_ctx = tc.tile_pool(name="mulw", bufs=2)
        mulw = mulw_ctx.__enter__()

        grp = 0
        with tc.tile_pool(name="ps_s", bufs=2, space="PSUM") as ps_s, \
             tc.tile_pool(name="ps_u", bufs=4, space="PSUM") as ps_u:
            us_by_head = {}

            def av_chunks(st):
                e2gs, nqb, h, gpair = st
                key = (nqb, h)
                if gpair == 0:
                    us = []
                    for _uq in range(4):
                        u_acc = ps_u.tile([128, D + 1], F32, tag="u")
                        us.append(u_acc)
                    us_by_head[key] = us
                us = us_by_head[key]
                e2g = e2gs[0]
                for j2 in range(4):
                    for jj in (2 * j2, 2 * j2 + 1):
                        mt = 8 * gpair + jj
                        for qs in range(4):
                            nc.tensor.matmul(
                                us[qs], e2g[:, jj, qs * 128:(qs + 1) * 128],
                                v_sb[:, mt, h, :],
                                start=(mt == 0), stop=(mt == NT - 1))
                    yield
                if gpair == 1:
                    for qs in range(4):
                        qt = nqb * 4 + qs
                        rec = work.tile([128, 1], F32, tag="rec")
                        nc.vector.reciprocal(rec, us[qs][:, D:D + 1])
                        nc.vector.tensor_scalar_mul(
                            o_sb[:, qt, h, :], us[qs][:, 0:D], rec)
                    del us_by_head[key]

            def emit_av(st):
                for _ in av_chunks(st):
                    pass

            def emit_proj(nqb):
                # proj for one query block; oT written by DMA transposes
                for ot in range(CT):
                    ps = ps_s.tile([128, 512], F32, tag="s")
                    for ct in range(CT):
                        nc.tensor.matmul(
                            ps, pw_sb[:, ct, ot * 128:(ot + 1) * 128],
                            oT_sb[:, ct, nqb * 512:(nqb + 1) * 512],
                            start=(ct == 0), stop=(ct == CT - 1))
                    fch = fin_pool.tile([128, 512], F32, tag="fin")
                    nc.vector.tensor_scalar_add(fch, ps, pbias[:, ot:ot + 1])
                    eng = (nc.sync, nc.gpsimd)[ot % 2]
                    eng.dma_start(outT_r[:, ot, nqb * 512:(nqb + 1) * 512],
                                  fch)

            def emit_bounce(bq):
                o_dview = o_dram.rearrange("(qt p) c -> p qt c", p=128)
                nc.sync.dma_start(o_dview[:, bq * 4:(bq + 1) * 4],
                                  o_sb[:, bq * 4:(bq + 1) * 4].rearrange(
                                      "p qt h d -> p qt (h d)"))
                for ct in range(CT):
                    # same queue as the o_dram write: FIFO order guarantees
                    # the bounce completes before the transpose reads it
                    nc.sync.dma_start_transpose(
                        oT_sb[:, ct, bq * 512:(bq + 1) * 512],
                        o_dram[bq * 512:(bq + 1) * 512,
                               ct * 128:(ct + 1) * 128])

            pending = []
            stage_i = 0
            for nqb in range(NQ // 512):
                for hp in range(H // 2):
                    kt = hp
                    for half in range(2):
                        rl, rh = half * 64, half * 64 + 64
                        for gpair in range(2):   # 8 mts per gpair
                            if stage_i < 3:
                                emit_v(stage_i + 1, ps_s, "s")
                            # ready av work from 5 stages back is emitted
                            # interleaved BEFORE each score group, so PE's
                            # in-order queue never parks ready avs behind
                            # slot-stalled score matmuls
                            av_gen = (av_chunks(pending.pop(0))
                                      if len(pending) > 4 else iter(()))
                            t2b = mulw.tile([128, 8, 512], BF16, tag="t2b")
                            for mt2 in range(4 * gpair, 4 * gpair + 4):
                                next(av_gen, None)
                                pa2 = pa[:, 2 * mt2:2 * mt2 + 2,
                                         nqb * 512:(nqb + 1) * 512]
                                s_ps = ps_s.tile([128, 2, 512], F32,
                                                 tag="s")
                                for j in range(2):
                                    mt = 2 * mt2 + j
                                    nc.tensor.matmul(
                                        s_ps[:, j],
                                        kT_sb[rl:rh, kt,
                                              mt * 128:(mt + 1) * 128],
                                        qT_sb[rl:rh, kt,
                                              nqb * 512:(nqb + 1) * 512],
                                        start=True, stop=True)
                                li = mt2 - 4 * gpair
                                tsl = t2b[:, 2 * li:2 * li + 2]
                                # route one mt2 of four via ACT-copy +
                                # GpSimd mul, concurrent with DVE's three
                                if li == 1 or (li == 3 and grp % 2 == 0):
                                    s8 = mulw.tile([128, 2, 512], BF16,
                                                   tag="s8")
                                    if (li + grp) % 2 == 0:
                                        nc.scalar.copy(s8, s_ps)
                                    else:
                                        nc.vector.tensor_copy(s8, s_ps)
                                    nc.gpsimd.tensor_mul(tsl, s8, pa2)
                                else:
                                    nc.vector.tensor_mul(tsl, s_ps, pa2)
                            grp += 1
                            for _ in av_gen:
                                pass
                            e2g = stream.tile([128, 8, 512], BF16,
                                              tag="xch")
                            nc.scalar.activation(e2g, t2b, AF.Exp,
                                                 scale=SCALE)
                            e2gs = [e2g]
                            pending.append((e2gs, nqb, 2 * hp + half, gpair))
                            stage_i += 1
                            if stage_i == 16 + 6:
                                # nq0's last av popped at stage 21
                                emit_bounce(0)
                            elif stage_i == 16 + 8:
                                emit_proj(0)
            while pending:
                emit_av(pending.pop(0))
            emit_bounce(NQ // 512 - 1)
            emit_proj(NQ // 512 - 1)
        mulw_ctx.__exit__(None, None, None)


_NC_CACHE = {}


def _get_nc():
    if "nc" not in _NC_CACHE:
        _NC_CACHE["nc"] = build()
    return _NC_CACHE["nc"]


def make_in_maps(x, pe, qkv_w, proj_w, proj_b, conv1_w, conv1_b, gn1_g, gn1_b,
                 conv2_w, conv2_b, gn2_g, gn2_b):
    f = np.float32
    bf = ml_dtypes.bfloat16
    shared = {
        "cw1": np.ascontiguousarray(np.asarray(conv1_w, f).T),
        "cw2": np.ascontiguousarray(np.asarray(conv2_w, f).T),
        "qw": np.ascontiguousarray(np.asarray(qkv_w, f).T),
        "pw": np.ascontiguousarray(np.asarray(proj_w, f).T).astype(bf),
        "cb1": np.asarray(conv1_b, f),
        "cb2": np.asarray(conv2_b, f),
        "gn1g": np.asarray(gn1_g, f),
        "gn1b": np.asarray(gn1_b, f),
        "gn2g": np.asarray(gn2_g, f),
        "gn2b": np.asarray(gn2_b, f),
        "pb": np.asarray(proj_b, f),
        "gmask": np.repeat(np.eye(2, dtype=f), 64, axis=0),
        "gmaskT": np.ascontiguousarray(np.repeat(np.eye(2, dtype=f), 64, axis=0).T),
        "vones": np.ones((128, NT * H), np.float32).astype(bf),
    }
    in_maps = []
    for c in range(N_CORES):
        b, h = c // 2, c % 2
        xT = np.asarray(x[b], f).T
        peT = np.asarray(pe[b], f).T
        if h == 1:
            xT = np.concatenate([xT[:, NQ:], xT[:, :NQ]], axis=1)
            peT = np.concatenate([peT[:, NQ:], peT[:, :NQ]], axis=1)
        m = dict(shared)
        m["xT"] = np.ascontiguousarray(xT)
        m["peT"] = np.ascontiguousarray(peT)
        in_maps.append(m)
    return in_maps


def assemble_out(results):
    B = N_CORES // 2
    out = np.empty((B, N, C), np.float32)
    for c in range(N_CORES):
        b, h = c // 2, c % 2
        out[b, h * NQ:(h + 1) * NQ, :] = results[c]["outT"].T
    return out


def kernel(**inputs):
    nc = _get_nc()
    in_maps = make_in_maps(**inputs)
    r = run_bass_kernel_spmd(nc, in_maps, core_ids=list(range(N_CORES)))
    return assemble_out(r.results)


if __name__ == "__main__":
    nc = build()
    print("build+compile OK")


# revision 12
# speedup vs baseline: 1.0261x; 1.0261x over previous
"""TRN2 Bass kernel for nn_Attention_87308095193383 — v5.

Sharding: 8 cores = (batch b in 0..3) x (query-half h in 0..1), SPMD with
host-permuted columns so "my queries" are always columns 0:1024.

Key structure (single core):
  A: conv1/conv2 (pe streamed nch-major) + GroupNorm (bn_stats, group
     reduce via indicator matmuls, affine as DVE 2x tensor_scalar)
  C: z = p2^T p1 -> sigmoid -> pa [keys, queries] bf16
  D: qkv, nch-major; x chunks + qkv_w sections prefetched through the
     shared rotating "stream" pool (slot-release sems give free prefetch)
  E: per (nqb, head): scores (f32r, K=64) -> mul by pa IN-PLACE in PSUM
     (DVE; some groups via ACT-copy + GpSimd mul) -> exp from PSUM (ACT
     -> bf16 e2, tiles also in the stream pool) -> attn@v accumulated in
     [q-partitions, 65-free] bf16; reciprocal([128,1]) + tensor_scalar
     normalize; depth-2 software pipeline so PE never waits on exp.
     o -> DRAM -> XBAR transpose back as oT [c, q]; proj for block 0
     interleaved into E of block 1.
  F: proj (bf16) + bias, streaming per-(ot,nqb) output DMA.
"""
import numpy as np
import ml_dtypes

import concourse.bass as bass
import concourse.mybir as mybir
import concourse.tile as tile
from concourse import bacc
from concourse.bass_utils import run_bass_kernel_spmd

F32R = mybir.dt.float32r
F32 = mybir.dt.float32
BF16 = mybir.dt.bfloat16
AF = mybir.ActivationFunctionType
ALU = mybir.AluOpType

N_CORES = 8
C = 512          # channels
CT = C // 128    # 4 c-tiles
N = 2048         # sequence length
NT = N // 128    # 16 m-tiles
NQ = 1024        # queries per core
H = 8            # heads
D = 64           # head dim
SCALE = D ** -0.5
EPS = 1e-5

# every k-th score group takes the ACT-copy + GpSimd-mul route
C_EVERY = 5


def build():
    nc = bacc.Bacc("TRN2", target_bir_lowering=False, debug=False,
                   num_devices=N_CORES)

    def din(name, shape, dt=F32R):
        return nc.dram_tensor(name, shape, dt, kind="ExternalInput").ap()

    peT = din("peT", [C, N])
    xT = din("xT", [C, N])
    cw1 = din("cw1", [C, C])        # conv1_w.T  [c_in, o]
    cw2 = din("cw2", [C, C])
    qw = din("qw", [C, 3 * C])      # qkv_w.T    [c_in, o]
    pw = din("pw", [C, C], BF16)    # proj_w.T
    cb1 = din("cb1", [C], F32)
    cb2 = din("cb2", [C], F32)
    gn1g = din("gn1g", [C], F32)
    gn1b = din("gn1b", [C], F32)
    gn2g = din("gn2g", [C], F32)
    gn2b = din("gn2b", [C], F32)
    pb = din("pb", [C], F32)
    gmask_in = din("gmask", [128, 2], F32)
    gmaskT_in = din("gmaskT", [2, 128], F32)
    vones_in = din("vones", [128, NT * H], BF16)
    outT = nc.dram_tensor("outT", [C, NQ], F32, kind="ExternalOutput").ap()

    with tile.TileContext(nc) as tc:
        _build_body(nc, tc, peT, xT, cw1, cw2, qw, pw, cb1, cb2,
                    gn1g, gn1b, gn2g, gn2b, pb, gmask_in, gmaskT_in,
                    vones_in, outT)
    nc.compile()
    return nc


def _build_body(nc, tc, peT, xT, cw1, cw2, qw, pw, cb1, cb2,
                gn1g, gn1b, gn2g, gn2b, pb, gmask_in, gmaskT_in,
                vones_in, outT):
    from contextlib import ExitStack
    ctx = ExitStack()
    with ctx:
        consts = ctx.enter_context(tc.tile_pool(name="consts", bufs=1))
        work = ctx.enter_context(tc.tile_pool(name="work", bufs=3))
        pa_pool = ctx.enter_context(tc.tile_pool(name="pa", bufs=1))
        pa = pa_pool.tile([128, NT, NQ], BF16)   # sigmoid(pe_attn) [k, q]
        pw_pool = ctx.enter_context(tc.tile_pool(name="pw_pool", bufs=1))
        pw_sb = pw_pool.tile([128, CT, C], BF16)
        # shared rotating slots: pe chunks -> x chunks / qw sections -> e2
        stream = ctx.enter_context(tc.tile_pool(name="stream", bufs=6))
        dram_pool = ctx.enter_context(tc.tile_pool(name="dscr", bufs=1,
                                                   space="DRAM"))
        o_dram = dram_pool.tile([NQ, C], BF16)

        # ---- constants (scalar queue, needed from ~25us)
        gmask = consts.tile([128, 2], F32)     # group-membership mask
        nc.scalar.dma_start(gmask, gmask_in)
        gmaskT = consts.tile([2, 128], F32)
        nc.scalar.dma_start(gmaskT, gmaskT_in)
        epst = consts.tile([128, 1], F32)
        nc.vector.memset(epst, EPS)
        bias1 = consts.tile([128, CT], F32)
        nc.scalar.dma_start(bias1, cb1.rearrange("(t p) -> p t", p=128))
        bias2 = consts.tile([128, CT], F32)
        nc.scalar.dma_start(bias2, cb2.rearrange("(t p) -> p t", p=128))
        g1g = consts.tile([128, CT], F32)
        nc.scalar.dma_start(g1g, gn1g.rearrange("(t p) -> p t", p=128))
        g1b = consts.tile([128, CT], F32)
        nc.scalar.dma_start(g1b, gn1b.rearrange("(t p) -> p t", p=128))
        g2g = consts.tile([128, CT], F32)
        nc.scalar.dma_start(g2g, gn2g.rearrange("(t p) -> p t", p=128))
        g2b = consts.tile([128, CT], F32)
        nc.scalar.dma_start(g2b, gn2b.rearrange("(t p) -> p t", p=128))
        pbias = consts.tile([128, CT], F32)
        nc.scalar.dma_start(pbias, pb.rearrange("(t p) -> p t", p=128))

        # warm the Sqrt ACT-table set during the DMA wait (Copy is in
        # every set; Sigmoid/Exp sets get pre-triggered later)
        warmt = consts.tile([128, 2], F32)
        nc.vector.memset(warmt, 0.0)
        nc.scalar.activation(warmt[:, 0:1], warmt[:, 0:1], AF.Sqrt)

        # ================= stage A/B: conv + groupnorm =================
        ps_abcd = tc.tile_pool(name="ps_mm", bufs=6, space="PSUM")
        ps_mm = ps_abcd.__enter__()
        with tc.tile_pool(name="cw_pool", bufs=1) as cw_pool, \
             tc.tile_pool(name="p12", bufs=1) as p12_pool:
            cw1_sb = cw_pool.tile([128, CT, C], F32R)
            nc.gpsimd.dma_start(cw1_sb, cw1.rearrange("(t p) o -> p t o", p=128))
            cw2_sb = cw_pool.tile([128, CT, C], F32R)
            nc.gpsimd.dma_start(cw2_sb, cw2.rearrange("(t p) o -> p t o", p=128))
            # pw is only needed at proj; keep it behind the conv weights
            nc.gpsimd.dma_start(pw_sb, pw.rearrange("(t p) o -> p t o", p=128))
            pe_r = peT.rearrange("(t p) n -> p t n", p=128)

            # p1 only needs its first NQ columns kept; p2 needs all N.
            p1_sb = p12_pool.tile([128, CT, NQ], F32R)
            p2_sb = p12_pool.tile([128, CT, N], F32R)

            convs = [(cw1_sb, bias1, g1g, g1b, p1_sb, NQ),
                     (cw2_sb, bias2, g2g, g2b, p2_sb, N)]
            statss = []
            for conv_i in range(2):
                stats = work.tile([128, CT, N // 512, 6], F32,
                                  tag=f"gnstats{conv_i}")
                statss.append(stats)
            # conv-major: conv1's GN chain overlaps conv2's matmuls.
            pe_chs = []
            for nch in range(N // 512):
                pe_ch = stream.tile([128, CT, 512], F32R, tag="xch")
                nc.sync.dma_start(pe_ch, pe_r[:, :, nch * 512:(nch + 1) * 512])
                pe_chs.append(pe_ch)
            for conv_i, (cwsb, cbt, gg, gb, dst, keep) in enumerate(convs):
                for nch in range(N // 512):
                    for ot in range(CT):
                        ps = ps_mm.tile([128, 512], F32, tag="mm")
                        for ct in range(CT):
                            nc.tensor.matmul(
                                ps, cwsb[:, ct, ot * 128:(ot + 1) * 128],
                                pe_chs[nch][:, ct],
                                start=(ct == 0), stop=(ct == CT - 1))
                        nc.vector.bn_stats(statss[conv_i][:, ot, nch], ps)
                        if nch * 512 < keep:
                            # keep-copies on ACT (DVE busy with bn_stats)
                            nc.scalar.copy(dst[:, ot, nch * 512:(nch + 1) * 512], ps)

            # x chunks + qw sections claim slots as pe chunks retire; the
            # DMAs issue from sync/gpsimd SEQs within the first ~15us, so
            # everything stage D needs is resident before it starts.
            x_r = xT.rearrange("(t p) n -> p t n", p=128)
            qw_r = qw.rearrange("(t p) o -> p t o", p=128)
            x_chs = []
            qsecs = []
            preload = [("x", 0, nc.scalar), ("x", 1, nc.scalar),
                       ("q", 1, nc.gpsimd), ("q", 0, nc.scalar),
                       ("q", 2, nc.scalar), ("x", 2, nc.gpsimd),
                       ("x", 3, nc.gpsimd)]
            for kind, i, eng in preload:
                if kind == "x":
                    t_x = stream.tile([128, CT, 512], F32R, tag="xch")
                    eng.dma_start(t_x, x_r[:, :, i * 512:(i + 1) * 512])
                    x_chs.append((i, t_x))
                else:
                    t_q = stream.tile([128, CT, 512], F32R, tag="xch")
                    eng.dma_start(t_q, qw_r[:, :, i * 512:(i + 1) * 512])
                    qsecs.append((i, t_q))
            x_chs = [t for _, t in sorted(x_chs)]
            qsecs = [t for _, t in sorted(qsecs)]

            for conv_i, (cwsb, cbt, gg, gb, dst, keep) in enumerate(convs):
                stats = statss[conv_i]
                mv2 = work.tile([128, 2, CT], F32, tag="gnmv")
                stack3 = work.tile([128, 3, CT], F32, tag="gnstack")
                for ot in range(CT):
                    nc.vector.bn_aggr(mv2[:, :, ot], stats[:, ot])
                nc.vector.tensor_add(stack3[:, 0], mv2[:, 0], cbt)
                nc.vector.tensor_copy(stack3[:, 1], mv2[:, 1])
                nc.vector.tensor_mul(stack3[:, 2], stack3[:, 0], stack3[:, 0])
                # group sums over 64-partition halves (all ots at once)
                gs = ps_mm.tile([2, 3, CT], F32, tag="mm")
                nc.tensor.matmul(gs, gmask, stack3.rearrange("p a t -> p (a t)"),
                                 start=True, stop=True)
                gss = work.tile([2, 3, CT], F32, tag="gss")
                nc.scalar.copy(gss, gs)
                gstat = work.tile([2, 2, CT], F32, tag="gstat")  # [mean, rstd]
                nc.vector.tensor_scalar_mul(gstat[:, 0], gss[:, 0], 1.0 / 64.0)
                vt = work.tile([2, 2, CT], F32, tag="gvtmp")
                nc.vector.tensor_add(vt[:, 0], gss[:, 1], gss[:, 2])
                nc.vector.tensor_scalar_mul(vt[:, 0], vt[:, 0], 1.0 / 64.0)
                nc.vector.tensor_mul(vt[:, 1], gstat[:, 0], gstat[:, 0])
                nc.vector.tensor_sub(vt[:, 0], vt[:, 0], vt[:, 1])
                nc.scalar.activation(vt[:, 0], vt[:, 0], AF.Sqrt, bias=epst[0:2])
                nc.vector.reciprocal(gstat[:, 1], vt[:, 0])
                # broadcast group [mean, rstd] to partitions via indicator MM
                bc_ps = ps_mm.tile([128, 2, CT], F32, tag="mm")
                nc.tensor.matmul(bc_ps, gmaskT,
                                 gstat.rearrange("p a t -> p (a t)"),
                                 start=True, stop=True)
                bcst = work.tile([128, 2, CT], F32, tag="gbc")
                nc.scalar.copy(bcst, bc_ps)
                # per-channel affine: y = x*sc + sh
                sc = work.tile([128, 2, CT], F32, tag=f"gsc{conv_i}")
                nc.vector.tensor_mul(sc[:, 0], bcst[:, 1], gg)
                nc.vector.tensor_sub(sc[:, 1], cbt, bcst[:, 0])
                nc.vector.tensor_mul(sc[:, 1], sc[:, 1], sc[:, 0])
                nc.vector.tensor_add(sc[:, 1], sc[:, 1], gb)
                for nch in range(keep // 512):
                    for ot in range(CT):
                        # DVE all-SBUF tensor_scalar runs in 2x mode
                        nc.vector.tensor_scalar(
                            dst[:, ot, nch * 512:(nch + 1) * 512],
                            dst[:, ot, nch * 512:(nch + 1) * 512],
                            sc[:, 0, ot:ot + 1], sc[:, 1, ot:ot + 1],
                            op0=ALU.mult, op1=ALU.add)

            # pre-trigger the Sigmoid table-set load off the critical path
            nc.scalar.activation(warmt[:, 1:2], warmt[:, 1:2], AF.Sigmoid)

            # ================= stage C: pe_attn = sigmoid(p2^T p1) =====
            for mt in range(NT):
                for nq in range(NQ // 512):
                    zps = ps_mm.tile([128, 512], F32, tag="mm")
                    for ct in range(CT):
                        nc.tensor.matmul(
                            zps, p2_sb[:, ct, mt * 128:(mt + 1) * 128],
                            p1_sb[:, ct, nq * 512:(nq + 1) * 512],
                            start=(ct == 0), stop=(ct == CT - 1))
                    nc.scalar.activation(pa[:, mt, nq * 512:(nq + 1) * 512],
                                         zps, AF.Sigmoid)

        # ================= stage D: qkv =================
        kqv_pool = ctx.enter_context(tc.tile_pool(name="kqv", bufs=1))
        kT_sb = kqv_pool.tile([128, CT, N], F32R)
        qT_sb = kqv_pool.tile([128, CT, NQ], F32R)
        v_sb = kqv_pool.tile([128, NT, H, D + 1], BF16)
        nc.sync.dma_start(
            v_sb[:, :, :, D:D + 1].rearrange("p t o u -> p (t o u)"),
            vones_in)

        def emit_v(nch, pool, tag):
            xc = x_chs[nch]
            for ntl in range(4):
                nt = nch * 4 + ntl
                ps = pool.tile([128, 512], F32, tag=tag)
                for ct in range(CT):
                    nc.tensor.matmul(
                        ps, xc[:, ct, ntl * 128:(ntl + 1) * 128],
                        qsecs[2][:, ct],
                        start=(ct == 0), stop=(ct == CT - 1))
                nc.vector.tensor_copy(v_sb[:, nt, :, 0:D],
                                      ps.rearrange("p (h d) -> p h d", h=H))

        # nch-major kq; x chunks stay live for the v matmuls that are
        # interleaved into early stage E (v0 mid-D to unblock x3's slot)
        for nch in range(N // 512):
            xc = x_chs[nch]
            for ot in range(CT):
                ps = ps_mm.tile([128, 512], F32, tag="mm")
                for ct in range(CT):
                    nc.tensor.matmul(
                        ps, qsecs[1][:, ct, ot * 128:(ot + 1) * 128],
                        xc[:, ct],
                        start=(ct == 0), stop=(ct == CT - 1))
                # split kT copies between ACT and DVE
                if nch % 2 == 0:
                    nc.scalar.copy(kT_sb[:, ot, nch * 512:(nch + 1) * 512], ps)
                else:
                    nc.vector.tensor_copy(kT_sb[:, ot, nch * 512:(nch + 1) * 512], ps)
            if nch < NQ // 512:
                for ot in range(CT):
                    ps = ps_mm.tile([128, 512], F32, tag="mm")
                    for ct in range(CT):
                        nc.tensor.matmul(
                            ps, qsecs[0][:, ct, ot * 128:(ot + 1) * 128],
                            xc[:, ct],
                            start=(ct == 0), stop=(ct == CT - 1))
                    nc.vector.tensor_copy(qT_sb[:, ot, nch * 512:(nch + 1) * 512], ps)
            if nch == 1:
                emit_v(0, ps_mm, "mm")
        ps_abcd.__exit__(None, None, None)

        # ================= stage E: attention =================
        o_pool = ctx.enter_context(tc.tile_pool(name="opool", bufs=1))
        o_sb = o_pool.tile([128, NQ // 128, H, D], BF16)   # [q, qt, h, d]
        oT_sb = o_pool.tile([128, CT, NQ], BF16)           # [c, ct, q]
        fin_pool = ctx.enter_context(tc.tile_pool(name="finp", bufs=4))
        outT_r = outT.rearrange("(t p) n -> p t n", p=128)
        mulw_ctx = tc.tile_pool(name="mulw", bufs=2)
        mulw = mulw_ctx.__enter__()

        grp = 0
        with tc.tile_pool(name="ps_s", bufs=2, space="PSUM") as ps_s, \
             tc.tile_pool(name="ps_u", bufs=4, space="PSUM") as ps_u:
            us_by_head = {}

            def av_chunks(st):
                e2gs, nqb, h, gpair = st
                key = (nqb, h)
                if gpair == 0:
                    us = []
                    for _uq in range(4):
                        u_acc = ps_u.tile([128, D + 1], F32, tag="u")
                        us.append(u_acc)
                    us_by_head[key] = us
                us = us_by_head[key]
                e2g = e2gs[0]
                for j2 in range(4):
                    for jj in (2 * j2, 2 * j2 + 1):
                        mt = 8 * gpair + jj
                        for qs in range(4):
                            nc.tensor.matmul(
                                us[qs], e2g[:, jj, qs * 128:(qs + 1) * 128],
                                v_sb[:, mt, h, :],
                                start=(mt == 0), stop=(mt == NT - 1))
                    yield
                if gpair == 1:
                    for qs in range(4):
                        qt = nqb * 4 + qs
                        rec = work.tile([128, 1], F32, tag="rec")
                        nc.vector.reciprocal(rec, us[qs][:, D:D + 1])
                        nc.vector.tensor_scalar_mul(
                            o_sb[:, qt, h, :], us[qs][:, 0:D], rec)
                    del us_by_head[key]

            def emit_av(st):
                for _ in av_chunks(st):
                    pass

            def emit_proj(nqb):
                # proj for one query block; oT written by DMA transposes
                for ot in range(CT):
                    ps = ps_s.tile([128, 512], F32, tag="s")
                    for ct in range(CT):
                        nc.tensor.matmul(
                            ps, pw_sb[:, ct, ot * 128:(ot + 1) * 128],
                            oT_sb[:, ct, nqb * 512:(nqb + 1) * 512],
                            start=(ct == 0), stop=(ct == CT - 1))
                    fch = fin_pool.tile([128, 512], F32, tag="fin")
                    nc.vector.tensor_scalar_add(fch, ps, pbias[:, ot:ot + 1])
                    eng = (nc.sync, nc.gpsimd)[ot % 2]
                    eng.dma_start(outT_r[:, ot, nqb * 512:(nqb + 1) * 512],
                                  fch)

            def emit_bounce(bq):
                o_dview = o_dram.rearrange("(qt p) c -> p qt c", p=128)
                nc.sync.dma_start(o_dview[:, bq * 4:(bq + 1) * 4],
                                  o_sb[:, bq * 4:(bq + 1) * 4].rearrange(
                                      "p qt h d -> p qt (h d)"))
                for ct in range(CT):
                    # same queue as the o_dram write: FIFO order guarantees
                    # the bounce completes before the transpose reads it
                    nc.sync.dma_start_transpose(
                        oT_sb[:, ct, bq * 512:(bq + 1) * 512],
                        o_dram[bq * 512:(bq + 1) * 512,
                               ct * 128:(ct + 1) * 128])

            pending = []
            stage_i = 0
            for nqb in range(NQ // 512):
                for hp in range(H // 2):
                    kt = hp
                    for half in range(2):
                        rl, rh = half * 64, half * 64 + 64
                        for gpair in range(2):   # 8 mts per gpair
                            if stage_i < 3:
                                emit_v(stage_i + 1, ps_s, "s")
                            # ready av work from 5 stages back is emitted
                            # interleaved BEFORE each score group, so PE's
                            # in-order queue never parks ready avs behind
                            # slot-stalled score matmuls
                            av_gen = (av_chunks(pending.pop(0))
                                      if len(pending) > 3 else iter(()))
                            t2b = mulw.tile([128, 8, 512], BF16, tag="t2b")
                            for mt2 in range(4 * gpair, 4 * gpair + 4):
                                next(av_gen, None)
                                pa2 = pa[:, 2 * mt2:2 * mt2 + 2,
                                         nqb * 512:(nqb + 1) * 512]
                                s_ps = ps_s.tile([128, 2, 512], F32,
                                                 tag="s")
                                for j in range(2):
                                    mt = 2 * mt2 + j
                                    nc.tensor.matmul(
                                        s_ps[:, j],
                                        kT_sb[rl:rh, kt,
                                              mt * 128:(mt + 1) * 128],
                                        qT_sb[rl:rh, kt,
                                              nqb * 512:(nqb + 1) * 512],
                                        start=True, stop=True)
                                li = mt2 - 4 * gpair
                                tsl = t2b[:, 2 * li:2 * li + 2]
                                # route one mt2 of four via ACT-copy +
                                # GpSimd mul, concurrent with DVE's three
                                if li == 1 or (li == 3 and grp % 2 == 0):
                                    s8 = mulw.tile([128, 2, 512], BF16,
                                                   tag="s8")
                                    if (li + grp) % 2 == 0:
                                        nc.scalar.copy(s8, s_ps)
                                    else:
                                        nc.vector.tensor_copy(s8, s_ps)
                                    nc.gpsimd.tensor_mul(tsl, s8, pa2)
                                else:
                                    nc.vector.tensor_mul(tsl, s_ps, pa2)
                            grp += 1
                            for _ in av_gen:
                                pass
                            e2g = stream.tile([128, 8, 512], BF16,
                                              tag="xch")
                            nc.scalar.activation(e2g, t2b, AF.Exp,
                                                 scale=SCALE)
                            e2gs = [e2g]
                            pending.append((e2gs, nqb, 2 * hp + half, gpair))
                            stage_i += 1
                            if stage_i == 16 + 6:
                                # nq0's last av popped at stage 21
                                emit_bounce(0)
                            elif stage_i == 16 + 8:
                                emit_proj(0)
            while pending:
                emit_av(pending.pop(0))
            emit_bounce(NQ // 512 - 1)
            emit_proj(NQ // 512 - 1)
        mulw_ctx.__exit__(None, None, None)


_NC_CACHE = {}


def _get_nc():
    if "nc" not in _NC_CACHE:
        _NC_CACHE["nc"] = build()
    return _NC_CACHE["nc"]


def make_in_maps(x, pe, qkv_w, proj_w, proj_b, conv1_w, conv1_b, gn1_g, gn1_b,
                 conv2_w, conv2_b, gn2_g, gn2_b):
    f = np.float32
    bf = ml_dtypes.bfloat16
    shared = {
        "cw1": np.ascontiguousarray(np.asarray(conv1_w, f).T),
        "cw2": np.ascontiguousarray(np.asarray(conv2_w, f).T),
        "qw": np.ascontiguousarray(np.asarray(qkv_w, f).T),
        "pw": np.ascontiguousarray(np.asarray(proj_w, f).T).astype(bf),
        "cb1": np.asarray(conv1_b, f),
        "cb2": np.asarray(conv2_b, f),
        "gn1g": np.asarray(gn1_g, f),
        "gn1b": np.asarray(gn1_b, f),
        "gn2g": np.asarray(gn2_g, f),
        "gn2b": np.asarray(gn2_b, f),
        "pb": np.asarray(proj_b, f),
        "gmask": np.repeat(np.eye(2, dtype=f), 64, axis=0),
        "gmaskT": np.ascontiguousarray(np.repeat(np.eye(2, dtype=f), 64, axis=0).T),
        "vones": np.ones((128, NT * H), np.float32).astype(bf),
    }
    in_maps = []
    for c in range(N_CORES):
        b, h = c // 2, c % 2
        xT = np.asarray(x[b], f).T
        peT = np.asarray(pe[b], f).T
        if h == 1:
            xT = np.concatenate([xT[:, NQ:], xT[:, :NQ]], axis=1)
            peT = np.concatenate([peT[:, NQ:], peT[:, :NQ]], axis=1)
        m = dict(shared)
        m["xT"] = np.ascontiguousarray(xT)
        m["peT"] = np.ascontiguousarray(peT)
        in_maps.append(m)
    return in_maps


def assemble_out(results):
    B = N_CORES // 2
    out = np.empty((B, N, C), np.float32)
    for c in range(N_CORES):
        b, h = c // 2, c % 2
        out[b, h * NQ:(h + 1) * NQ, :] = results[c]["outT"].T
    return out


def kernel(**inputs):
    nc = _get_nc()
    in_maps = make_in_maps(**inputs)
    r = run_bass_kernel_spmd(nc, in_maps, core_ids=list(range(N_CORES)))
    return assemble_out(r.results)


if __name__ == "__main__":
    nc = build()
    print("build+compile OK")


# revision 18
# speedup vs baseline: 1.0512x; 1.0244x over previous
"""TRN2 Bass kernel for nn_Attention_87308095193383 — v5.

Sharding: 8 cores = (batch b in 0..3) x (query-half h in 0..1), SPMD with
host-permuted columns so "my queries" are always columns 0:1024.

Key structure (single core):
  A: conv1/conv2 (pe streamed nch-major) + GroupNorm (bn_stats, group
     reduce via indicator matmuls, affine as DVE 2x tensor_scalar)
  C: z = p2^T p1 -> sigmoid -> pa [keys, queries] bf16
  D: qkv, nch-major; x chunks + qkv_w sections prefetched through the
     shared rotating "stream" pool (slot-release sems give free prefetch)
  E: per (nqb, head): scores (f32r, K=64) -> mul by pa IN-PLACE in PSUM
     (DVE; some groups via ACT-copy + GpSimd mul) -> exp from PSUM (ACT
     -> bf16 e2, tiles also in the stream pool) -> attn@v accumulated in
     [q-partitions, 65-free] bf16; reciprocal([128,1]) + tensor_scalar
     normalize; depth-2 software pipeline so PE never waits on exp.
     o -> DRAM -> XBAR transpose back as oT [c, q]; proj for block 0
     interleaved into E of block 1.
  F: proj (bf16) + bias, streaming per-(ot,nqb) output DMA.
"""
import numpy as np
import ml_dtypes

import concourse.bass as bass
import concourse.mybir as mybir
import concourse.tile as tile
from concourse import bacc
from concourse.bass_utils import run_bass_kernel_spmd

F32R = mybir.dt.float32r
F32 = mybir.dt.float32
BF16 = mybir.dt.bfloat16
AF = mybir.ActivationFunctionType
ALU = mybir.AluOpType

N_CORES = 8
C = 512          # channels
CT = C // 128    # 4 c-tiles
N = 2048         # sequence length
NT = N // 128    # 16 m-tiles
NQ = 1024        # queries per core
H = 8            # heads
D = 64           # head dim
SCALE = D ** -0.5
EPS = 1e-5

# every k-th score group takes the ACT-copy + GpSimd-mul route
C_EVERY = 5


def build():
    nc = bacc.Bacc("TRN2", target_bir_lowering=False, debug=False,
                   num_devices=N_CORES)

    def din(name, shape, dt=F32R):
        return nc.dram_tensor(name, shape, dt, kind="ExternalInput").ap()

    peT = din("peT", [C, N])
    xT = din("xT", [C, N])
    cw1 = din("cw1", [C, C])        # conv1_w.T  [c_in, o]
    cw2 = din("cw2", [C, C])
    qw = din("qw", [C, 3 * C])      # qkv_w.T    [c_in, o]
    pw = din("pw", [C, C], BF16)    # proj_w.T
    cb1 = din("cb1", [C], F32)
    cb2 = din("cb2", [C], F32)
    gn1g = din("gn1g", [C], F32)
    gn1b = din("gn1b", [C], F32)
    gn2g = din("gn2g", [C], F32)
    gn2b = din("gn2b", [C], F32)
    pb = din("pb", [C], F32)
    gmask_in = din("gmask", [128, 2], F32)
    gmaskT_in = din("gmaskT", [2, 128], F32)
    vones_in = din("vones", [128, NT * H], BF16)
    outT = nc.dram_tensor("outT", [C, NQ], F32, kind="ExternalOutput").ap()

    with tile.TileContext(nc) as tc:
        _build_body(nc, tc, peT, xT, cw1, cw2, qw, pw, cb1, cb2,
                    gn1g, gn1b, gn2g, gn2b, pb, gmask_in, gmaskT_in,
                    vones_in, outT)
    nc.compile()
    return nc


def _build_body(nc, tc, peT, xT, cw1, cw2, qw, pw, cb1, cb2,
                gn1g, gn1b, gn2g, gn2b, pb, gmask_in, gmaskT_in,
                vones_in, outT):
    from contextlib import ExitStack
    ctx = ExitStack()
    with ctx:
        consts = ctx.enter_context(tc.tile_pool(name="consts", bufs=1))
        work = ctx.enter_context(tc.tile_pool(name="work", bufs=3))
        pa_pool = ctx.enter_context(tc.tile_pool(name="pa", bufs=1))
        pa = pa_pool.tile([128, NT, NQ], BF16)   # sigmoid(pe_attn) [k, q]
        pw_pool = ctx.enter_context(tc.tile_pool(name="pw_pool", bufs=1))
        pw_sb = pw_pool.tile([128, CT, C], BF16)
        # shared rotating slots: pe chunks -> x chunks / qw sections -> e2
        stream = ctx.enter_context(tc.tile_pool(name="stream", bufs=6))
        dram_pool = ctx.enter_context(tc.tile_pool(name="dscr", bufs=1,
                                                   space="DRAM"))
        o_dram = dram_pool.tile([NQ, C], BF16)

        # ---- constants (scalar queue, needed from ~25us)
        gmask = consts.tile([128, 2], F32)     # group-membership mask
        nc.scalar.dma_start(gmask, gmask_in)
        gmaskT = consts.tile([2, 128], F32)
        nc.scalar.dma_start(gmaskT, gmaskT_in)
        epst = consts.tile([128, 1], F32)
        nc.vector.memset(epst, EPS)
        bias1 = consts.tile([128, CT], F32)
        nc.scalar.dma_start(bias1, cb1.rearrange("(t p) -> p t", p=128))
        bias2 = consts.tile([128, CT], F32)
        nc.scalar.dma_start(bias2, cb2.rearrange("(t p) -> p t", p=128))
        g1g = consts.tile([128, CT], F32)
        nc.scalar.dma_start(g1g, gn1g.rearrange("(t p) -> p t", p=128))
        g1b = consts.tile([128, CT], F32)
        nc.scalar.dma_start(g1b, gn1b.rearrange("(t p) -> p t", p=128))
        g2g = consts.tile([128, CT], F32)
        nc.scalar.dma_start(g2g, gn2g.rearrange("(t p) -> p t", p=128))
        g2b = consts.tile([128, CT], F32)
        nc.scalar.dma_start(g2b, gn2b.rearrange("(t p) -> p t", p=128))
        pbias = consts.tile([128, CT], F32)
        nc.scalar.dma_start(pbias, pb.rearrange("(t p) -> p t", p=128))

        # warm the Sqrt ACT-table set during the DMA wait (Copy is in
        # every set; Sigmoid/Exp sets get pre-triggered later)
        warmt = consts.tile([128, 2], F32)
        nc.vector.memset(warmt, 0.0)
        nc.scalar.activation(warmt[:, 0:1], warmt[:, 0:1], AF.Sqrt)

        # ================= stage A/B: conv + groupnorm =================
        ps_abcd = tc.tile_pool(name="ps_mm", bufs=6, space="PSUM")
        ps_mm = ps_abcd.__enter__()
        with tc.tile_pool(name="cw_pool", bufs=1) as cw_pool, \
             tc.tile_pool(name="p12", bufs=1) as p12_pool:
            cw1_sb = cw_pool.tile([128, CT, C], F32R)
            nc.gpsimd.dma_start(cw1_sb, cw1.rearrange("(t p) o -> p t o", p=128))
            cw2_sb = cw_pool.tile([128, CT, C], F32R)
            nc.gpsimd.dma_start(cw2_sb, cw2.rearrange("(t p) o -> p t o", p=128))
            # pw is only needed at proj; keep it behind the conv weights
            nc.gpsimd.dma_start(pw_sb, pw.rearrange("(t p) o -> p t o", p=128))
            pe_r = peT.rearrange("(t p) n -> p t n", p=128)

            # p1 only needs its first NQ columns kept; p2 needs all N.
            p1_sb = p12_pool.tile([128, CT, NQ], F32R)
            p2_sb = p12_pool.tile([128, CT, N], F32R)

            convs = [(cw1_sb, bias1, g1g, g1b, p1_sb, NQ),
                     (cw2_sb, bias2, g2g, g2b, p2_sb, N)]
            statss = []
            for conv_i in range(2):
                stats = work.tile([128, CT, N // 512, 6], F32,
                                  tag=f"gnstats{conv_i}")
                statss.append(stats)
            # conv-major: conv1's GN chain overlaps conv2's matmuls.
            pe_chs = []
            for nch in range(N // 512):
                pe_ch = stream.tile([128, CT, 512], F32R, tag="xch")
                nc.sync.dma_start(pe_ch, pe_r[:, :, nch * 512:(nch + 1) * 512])
                pe_chs.append(pe_ch)
            for conv_i, (cwsb, cbt, gg, gb, dst, keep) in enumerate(convs):
                for nch in range(N // 512):
                    for ot in range(CT):
                        ps = ps_mm.tile([128, 512], F32, tag="mm")
                        for ct in range(CT):
                            nc.tensor.matmul(
                                ps, cwsb[:, ct, ot * 128:(ot + 1) * 128],
                                pe_chs[nch][:, ct],
                                start=(ct == 0), stop=(ct == CT - 1))
                        nc.vector.bn_stats(statss[conv_i][:, ot, nch], ps)
                        if nch * 512 < keep:
                            # keep-copies on ACT (DVE busy with bn_stats)
                            nc.scalar.copy(dst[:, ot, nch * 512:(nch + 1) * 512], ps)

            # x chunks + qw sections claim slots as pe chunks retire; the
            # DMAs issue from sync/gpsimd SEQs within the first ~15us, so
            # everything stage D needs is resident before it starts.
            x_r = xT.rearrange("(t p) n -> p t n", p=128)
            qw_r = qw.rearrange("(t p) o -> p t o", p=128)
            x_chs = []
            qsecs = []
            preload = [("x", 0, nc.scalar), ("x", 1, nc.scalar),
                       ("q", 1, nc.gpsimd), ("q", 0, nc.scalar),
                       ("q", 2, nc.scalar), ("x", 2, nc.gpsimd),
                       ("x", 3, nc.gpsimd)]
            for kind, i, eng in preload:
                if kind == "x":
                    t_x = stream.tile([128, CT, 512], F32R, tag="xch")
                    eng.dma_start(t_x, x_r[:, :, i * 512:(i + 1) * 512])
                    x_chs.append((i, t_x))
                else:
                    t_q = stream.tile([128, CT, 512], F32R, tag="xch")
                    eng.dma_start(t_q, qw_r[:, :, i * 512:(i + 1) * 512])
                    qsecs.append((i, t_q))
            x_chs = [t for _, t in sorted(x_chs)]
            qsecs = [t for _, t in sorted(qsecs)]

            for conv_i, (cwsb, cbt, gg, gb, dst, keep) in enumerate(convs):
                stats = statss[conv_i]
                mv2 = work.tile([128, 2, CT], F32, tag="gnmv")
                stack3 = work.tile([128, 3, CT], F32, tag="gnstack")
                for ot in range(CT):
                    nc.vector.bn_aggr(mv2[:, :, ot], stats[:, ot])
                nc.vector.tensor_add(stack3[:, 0], mv2[:, 0], cbt)
                nc.vector.tensor_copy(stack3[:, 1], mv2[:, 1])
                nc.vector.tensor_mul(stack3[:, 2], stack3[:, 0], stack3[:, 0])
                # group sums over 64-partition halves (all ots at once)
                gs = ps_mm.tile([2, 3, CT], F32, tag="mm")
                nc.tensor.matmul(gs, gmask, stack3.rearrange("p a t -> p (a t)"),
                                 start=True, stop=True)
                gss = work.tile([2, 3, CT], F32, tag="gss")
                nc.scalar.copy(gss, gs)
                gstat = work.tile([2, 2, CT], F32, tag="gstat")  # [mean, rstd]
                nc.vector.tensor_scalar_mul(gstat[:, 0], gss[:, 0], 1.0 / 64.0)
                vt = work.tile([2, 2, CT], F32, tag="gvtmp")
                nc.vector.tensor_add(vt[:, 0], gss[:, 1], gss[:, 2])
                nc.vector.tensor_scalar_mul(vt[:, 0], vt[:, 0], 1.0 / 64.0)
                nc.vector.tensor_mul(vt[:, 1], gstat[:, 0], gstat[:, 0])
                nc.vector.tensor_sub(vt[:, 0], vt[:, 0], vt[:, 1])
                nc.scalar.activation(vt[:, 0], vt[:, 0], AF.Sqrt, bias=epst[0:2])
                nc.vector.reciprocal(gstat[:, 1], vt[:, 0])
                # broadcast group [mean, rstd] to partitions via indicator MM
                bc_ps = ps_mm.tile([128, 2, CT], F32, tag="mm")
                nc.tensor.matmul(bc_ps, gmaskT,
                                 gstat.rearrange("p a t -> p (a t)"),
                                 start=True, stop=True)
                bcst = work.tile([128, 2, CT], F32, tag="gbc")
                nc.scalar.copy(bcst, bc_ps)
                # per-channel affine: y = x*sc + sh
                sc = work.tile([128, 2, CT], F32, tag=f"gsc{conv_i}")
                nc.vector.tensor_mul(sc[:, 0], bcst[:, 1], gg)
                nc.vector.tensor_sub(sc[:, 1], cbt, bcst[:, 0])
                nc.vector.tensor_mul(sc[:, 1], sc[:, 1], sc[:, 0])
                nc.vector.tensor_add(sc[:, 1], sc[:, 1], gb)
                for nch in range(keep // 512):
                    for ot in range(CT):
                        # p2's first chunk gates stage C: run those 4 ops on
                        # the empty GpSimd queue so z starts without waiting
                        # DVE's bn_stats backlog; the rest stay on DVE's 2x
                        # all-SBUF fast path
                        eng = (nc.gpsimd if conv_i == 1 and nch == 0
                               else nc.vector)
                        eng.tensor_scalar(
                            dst[:, ot, nch * 512:(nch + 1) * 512],
                            dst[:, ot, nch * 512:(nch + 1) * 512],
                            sc[:, 0, ot:ot + 1], sc[:, 1, ot:ot + 1],
                            op0=ALU.mult, op1=ALU.add)

            # pre-trigger the Sigmoid table-set load off the critical path
            nc.scalar.activation(warmt[:, 1:2], warmt[:, 1:2], AF.Sigmoid)

            # ================= stage C: pe_attn = sigmoid(p2^T p1) =====
            for mt in range(NT):
                for nq in range(NQ // 512):
                    zps = ps_mm.tile([128, 512], F32, tag="mm")
                    for ct in range(CT):
                        nc.tensor.matmul(
                            zps, p2_sb[:, ct, mt * 128:(mt + 1) * 128],
                            p1_sb[:, ct, nq * 512:(nq + 1) * 512],
                            start=(ct == 0), stop=(ct == CT - 1))
                    nc.scalar.activation(pa[:, mt, nq * 512:(nq + 1) * 512],
                                         zps, AF.Sigmoid)

        # ================= stage D: qkv =================
        kqv_pool = ctx.enter_context(tc.tile_pool(name="kqv", bufs=1))
        kT_sb = kqv_pool.tile([128, CT, N], F32R)
        qT_sb = kqv_pool.tile([128, CT, NQ], F32R)
        v_sb = kqv_pool.tile([128, NT, H, D + 1], BF16)
        nc.sync.dma_start(
            v_sb[:, :, :, D:D + 1].rearrange("p t o u -> p (t o u)"),
            vones_in)

        def emit_v(nch, pool, tag):
            xc = x_chs[nch]
            for ntl in range(4):
                nt = nch * 4 + ntl
                ps = pool.tile([128, 512], F32, tag=tag)
                for ct in range(CT):
                    nc.tensor.matmul(
                        ps, xc[:, ct, ntl * 128:(ntl + 1) * 128],
                        qsecs[2][:, ct],
                        start=(ct == 0), stop=(ct == CT - 1))
                nc.vector.tensor_copy(v_sb[:, nt, :, 0:D],
                                      ps.rearrange("p (h d) -> p h d", h=H))

        # nch-major kq; x chunks stay live for the v matmuls that are
        # interleaved into early stage E (v0 mid-D to unblock x3's slot)
        for nch in range(N // 512):
            xc = x_chs[nch]
            for ot in range(CT):
                ps = ps_mm.tile([128, 512], F32, tag="mm")
                for ct in range(CT):
                    nc.tensor.matmul(
                        ps, qsecs[1][:, ct, ot * 128:(ot + 1) * 128],
                        xc[:, ct],
                        start=(ct == 0), stop=(ct == CT - 1))
                # split kT copies between ACT and DVE
                if nch % 2 == 0:
                    nc.scalar.copy(kT_sb[:, ot, nch * 512:(nch + 1) * 512], ps)
                else:
                    nc.vector.tensor_copy(kT_sb[:, ot, nch * 512:(nch + 1) * 512], ps)
            if nch < NQ // 512:
                for ot in range(CT):
                    ps = ps_mm.tile([128, 512], F32, tag="mm")
                    for ct in range(CT):
                        nc.tensor.matmul(
                            ps, qsecs[0][:, ct, ot * 128:(ot + 1) * 128],
                            xc[:, ct],
                            start=(ct == 0), stop=(ct == CT - 1))
                    nc.vector.tensor_copy(qT_sb[:, ot, nch * 512:(nch + 1) * 512], ps)
            if nch == 1:
                emit_v(0, ps_mm, "mm")
        ps_abcd.__exit__(None, None, None)

        # ================= stage E: attention =================
        o_pool = ctx.enter_context(tc.tile_pool(name="opool", bufs=1))
        o_sb = o_pool.tile([128, NQ // 128, H, D], BF16)   # [q, qt, h, d]
        oT_sb = o_pool.tile([128, CT, NQ], BF16)           # [c, ct, q]
        fin_pool = ctx.enter_context(tc.tile_pool(name="finp", bufs=2))
        outT_r = outT.rearrange("(t p) n -> p t n", p=128)
        mulw_ctx = tc.tile_pool(name="mulw", bufs=3)
        mulw = mulw_ctx.__enter__()

        grp = 0
        with tc.tile_pool(name="ps_s", bufs=2, space="PSUM") as ps_s, \
             tc.tile_pool(name="ps_u", bufs=4, space="PSUM") as ps_u:
            us_by_head = {}

            def av_chunks(st):
                e2gs, nqb, h, gpair = st
                key = (nqb, h)
                if gpair == 0:
                    us = []
                    for _uq in range(4):
                        u_acc = ps_u.tile([128, D + 1], F32, tag="u")
                        us.append(u_acc)
                    us_by_head[key] = us
                us = us_by_head[key]
                e2g = e2gs[0]
                for j2 in range(4):
                    for jj in (2 * j2, 2 * j2 + 1):
                        mt = 8 * gpair + jj
                        for qs in range(4):
                            nc.tensor.matmul(
                                us[qs], e2g[:, jj, qs * 128:(qs + 1) * 128],
                                v_sb[:, mt, h, :],
                                start=(mt == 0), stop=(mt == NT - 1))
                    yield
                if gpair == 1:
                    for qs in range(4):
                        qt = nqb * 4 + qs
                        rec = work.tile([128, 1], F32, tag="rec")
                        nc.vector.reciprocal(rec, us[qs][:, D:D + 1])
                        nc.vector.tensor_scalar_mul(
                            o_sb[:, qt, h, :], us[qs][:, 0:D], rec)
                    del us_by_head[key]

            def emit_av(st):
                for _ in av_chunks(st):
                    pass

            def emit_proj(nqb):
                # proj for one query block; oT written by DMA transposes
                for ot in range(CT):
                    ps = ps_s.tile([128, 512], F32, tag="s")
                    for ct in range(CT):
                        nc.tensor.matmul(
                            ps, pw_sb[:, ct, ot * 128:(ot + 1) * 128],
                            oT_sb[:, ct, nqb * 512:(nqb + 1) * 512],
                            start=(ct == 0), stop=(ct == CT - 1))
                    fch = fin_pool.tile([128, 512], F32, tag="fin")
                    nc.vector.tensor_scalar_add(fch, ps, pbias[:, ot:ot + 1])
                    eng = (nc.sync, nc.gpsimd)[ot % 2]
                    eng.dma_start(outT_r[:, ot, nqb * 512:(nqb + 1) * 512],
                                  fch)

            def emit_bounce(bq):
                o_dview = o_dram.rearrange("(qt p) c -> p qt c", p=128)
                nc.sync.dma_start(o_dview[:, bq * 4:(bq + 1) * 4],
                                  o_sb[:, bq * 4:(bq + 1) * 4].rearrange(
                                      "p qt h d -> p qt (h d)"))
                for ct in range(CT):
                    # same queue as the o_dram write: FIFO order guarantees
                    # the bounce completes before the transpose reads it
                    nc.sync.dma_start_transpose(
                        oT_sb[:, ct, bq * 512:(bq + 1) * 512],
                        o_dram[bq * 512:(bq + 1) * 512,
                               ct * 128:(ct + 1) * 128])

            pending = []
            stage_i = 0
            for nqb in range(NQ // 512):
                for hp in range(H // 2):
                    kt = hp
                    for half in range(2):
                        rl, rh = half * 64, half * 64 + 64
                        for gpair in range(2):   # 8 mts per gpair
                            if stage_i < 3:
                                emit_v(stage_i + 1, ps_s, "s")
                            # ready av work from 5 stages back is emitted
                            # interleaved BEFORE each score group, so PE's
                            # in-order queue never parks ready avs behind
                            # slot-stalled score matmuls
                            av_gen = (av_chunks(pending.pop(0))
                                      if len(pending) > 3 else iter(()))
                            t2b = mulw.tile([128, 8, 512], BF16, tag="t2b")
                            for mt2 in range(4 * gpair, 4 * gpair + 4):
                                next(av_gen, None)
                                pa2 = pa[:, 2 * mt2:2 * mt2 + 2,
                                         nqb * 512:(nqb + 1) * 512]
                                s_ps = ps_s.tile([128, 2, 512], F32,
                                                 tag="s")
                                for j in range(2):
                                    mt = 2 * mt2 + j
                                    nc.tensor.matmul(
                                        s_ps[:, j],
                                        kT_sb[rl:rh, kt,
                                              mt * 128:(mt + 1) * 128],
                                        qT_sb[rl:rh, kt,
                                              nqb * 512:(nqb + 1) * 512],
                                        start=True, stop=True)
                                li = mt2 - 4 * gpair
                                tsl = t2b[:, 2 * li:2 * li + 2]
                                # route one mt2 of four via ACT-copy +
                                # GpSimd mul, concurrent with DVE's three
                                if li == 1 or (li == 3 and grp % 2 == 0):
                                    s8 = mulw.tile([128, 2, 512], BF16,
                                                   tag="s8")
                                    if (li + grp) % 2 == 0:
                                        nc.scalar.copy(s8, s_ps)
                                    else:
                                        nc.vector.tensor_copy(s8, s_ps)
                                    nc.gpsimd.tensor_mul(tsl, s8, pa2)
                                else:
                                    nc.vector.tensor_mul(tsl, s_ps, pa2)
                            grp += 1
                            for _ in av_gen:
                                pass
                            e2g = stream.tile([128, 8, 512], BF16,
                                              tag="xch")
                            nc.scalar.activation(e2g, t2b, AF.Exp,
                                                 scale=SCALE)
                            e2gs = [e2g]
                            pending.append((e2gs, nqb, 2 * hp + half, gpair))
                            stage_i += 1
                            if stage_i == 16 + 6:
                                # nq0's last av popped at stage 21
                                emit_bounce(0)
                            elif stage_i == 16 + 8:
                                emit_proj(0)
            while pending:
                emit_av(pending.pop(0))
            emit_bounce(NQ // 512 - 1)
            emit_proj(NQ // 512 - 1)
        mulw_ctx.__exit__(None, None, None)


_NC_CACHE = {}


def _get_nc():
    if "nc" not in _NC_CACHE:
        _NC_CACHE["nc"] = build()
    return _NC_CACHE["nc"]


def make_in_maps(x, pe, qkv_w, proj_w, proj_b, conv1_w, conv1_b, gn1_g, gn1_b,
                 conv2_w, conv2_b, gn2_g, gn2_b):
    f = np.float32
    bf = ml_dtypes.bfloat16
    shared = {
        "cw1": np.ascontiguousarray(np.asarray(conv1_w, f).T),
        "cw2": np.ascontiguousarray(np.asarray(conv2_w, f).T),
        "qw": np.ascontiguousarray(np.asarray(qkv_w, f).T),
        "pw": np.ascontiguousarray(np.asarray(proj_w, f).T).astype(bf),
        "cb1": np.asarray(conv1_b, f),
        "cb2": np.asarray(conv2_b, f),
        "gn1g": np.asarray(gn1_g, f),
        "gn1b": np.asarray(gn1_b, f),
        "gn2g": np.asarray(gn2_g, f),
        "gn2b": np.asarray(gn2_b, f),
        "pb": np.asarray(proj_b, f),
        "gmask": np.repeat(np.eye(2, dtype=f), 64, axis=0),
        "gmaskT": np.ascontiguousarray(np.repeat(np.eye(2, dtype=f), 64, axis=0).T),
        "vones": np.ones((128, NT * H), np.float32).astype(bf),
    }
    in_maps = []
    for c in range(N_CORES):
        b, h = c // 2, c % 2
        xT = np.asarray(x[b], f).T
        peT = np.asarray(pe[b], f).T
        if h == 1:
            xT = np.concatenate([xT[:, NQ:], xT[:, :NQ]], axis=1)
            peT = np.concatenate([peT[:, NQ:], peT[:, :NQ]], axis=1)
        m = dict(shared)
        m["xT"] = np.ascontiguousarray(xT)
        m["peT"] = np.ascontiguousarray(peT)
        in_maps.append(m)
    return in_maps


def assemble_out(results):
    B = N_CORES // 2
    out = np.empty((B, N, C), np.float32)
    for c in range(N_CORES):
        b, h = c // 2, c % 2
        out[b, h * NQ:(h + 1) * NQ, :] = results[c]["outT"].T
    return out


def kernel(**inputs):
    nc = _get_nc()
    in_maps = make_in_maps(**inputs)
    r = run_bass_kernel_spmd(nc, in_maps, core_ids=list(range(N_CORES)))
    return assemble_out(r.results)


if __name__ == "__main__":
    nc = build()
    print("build+compile OK")
